# revision 1
# baseline (speedup 1.0000x reference)
import numpy as np
import concourse.bass as bass
import concourse.mybir as mybir
import concourse.tile as tile
from concourse.bass_utils import run_bass_kernel_spmd

F32 = mybir.dt.float32
F16 = mybir.dt.float16
AF = mybir.ActivationFunctionType
ALU = mybir.AluOpType

N = 8192
I = 512
O = 512
G = 8
NCORES = 8
RPC = N // NCORES          # 1024 rows per core
IB = I // 128              # 4 input-dim tiles
OT = O // 128              # 4 output-dim tiles
NH = RPC // 512            # 2 row halves (matmul moving free dim <= 512)
KT = IB * G * 2            # 64 weight k-tiles
MAGIC = 1.5 * 2.0**23
INV2PI = 1.0 / (2.0 * np.pi)
HALFPI = float(np.pi / 2)


def _split_multiwaits(nc):
    # ISA allows one sem-wait per instruction; TileContext's tail drain emits
    # several. Peel extras onto single-wait NoOps.
    n = 0
    for blk in nc.cur_f.blocks:
        insts = blk.instructions
        i = 0
        while i < len(insts):
            inst = insts[i]
            si = inst.sync_info
            if si is not None and len(si.on_wait) > 1:
                waits = list(si.on_wait)
                si.on_wait = [waits[-1]]
                for j, w in enumerate(waits[:-1]):
                    nop = mybir.InstNoOp(
                        name=f"I-waitsplit-{n}", engine=inst.engine, ins=[], outs=[],
                        sync_info=mybir.SyncInfo(on_wait=[w], on_update=[]))
                    n += 1
                    nc.register_instruction(nop)
                    insts.insert(i + j, nop)
                i += len(waits) - 1
            i += 1
    return n


def build():
    nc = bass.Bass()
    xT = nc.dram_tensor("xT", [I, RPC], F32, kind="ExternalInput")
    w = nc.dram_tensor("w", [KT, 128, O], F16, kind="ExternalInput")
    biasd = nc.dram_tensor("biasd", [OT, 128, 1], F32, kind="ExternalInput")
    yT = nc.dram_tensor("yT", [O, RPC], F32, kind="ExternalOutput")

    with tile.TileContext(nc) as tc:
        with tc.tile_pool(name="res", bufs=1) as rp, \
             tc.tile_pool(name="wp", bufs=4) as wp, \
             tc.tile_pool(name="dp", bufs=2) as dp, \
             tc.tile_pool(name="fp", bufs=4) as fp, \
             tc.tile_pool(name="yp", bufs=2) as yp, \
             tc.tile_pool(name="ps", bufs=1, space="PSUM") as pp:

            halfpi = rp.tile([128, 1], F32, name="halfpi")
            nc.vector.memset(halfpi[:], HALFPI)

            xt = []
            for ib in range(IB):
                xi = rp.tile([128, RPC], F32, name=f"x{ib}")
                nc.gpsimd.dma_start(xi[:], xT[ib * 128:(ib + 1) * 128, :])
                xt.append(xi)

            bt = []
            for ot in range(OT):
                bi = rp.tile([128, 1], F32, name=f"b{ot}")
                nc.gpsimd.dma_start(bi[:], biasd[ot])
                bt.append(bi)

            u1 = []
            for ib in range(IB):
                ui = rp.tile([128, RPC], F32, name=f"u{ib}")
                nc.vector.tensor_scalar_mul(ui[:], xt[ib][:], float(INV2PI))
                u1.append(ui)

            ps = [pp.tile([128, 512], F32, name=f"ps{i}") for i in range(OT * NH)]

            pair = 0
            npairs = IB * G
            for ib in range(IB):
                for gi in range(G):
                    g = float(gi + 1)
                    pg = 2.0 * np.pi / g
                    kt0 = (ib * G + gi) * 2

                    wc = wp.tile([128, O], F16, name="wc")
                    nc.sync.dma_start(wc[:], w[kt0])
                    ws = wp.tile([128, O], F16, name="ws")
                    nc.sync.dma_start(ws[:], w[kt0 + 1])

                    t = dp.tile([128, RPC], F32, name="t")
                    nc.vector.tensor_scalar(t[:], u1[ib][:], g, float(MAGIC),
                                            ALU.mult, ALU.add)
                    kf = dp.tile([128, RPC], F32, name="kf")
                    nc.vector.tensor_scalar_sub(kf[:], t[:], float(MAGIC))
                    rho = dp.tile([128, RPC], F32, name="rho")
                    nc.vector.scalar_tensor_tensor(rho[:], kf[:], float(-pg),
                                                   xt[ib][:], ALU.mult, ALU.add)
                    ar = dp.tile([128, RPC], F32, name="ar")
                    nc.scalar.activation(ar[:], rho[:], AF.Abs)
                    fs = fp.tile([128, RPC], F16, name="fs")
                    nc.scalar.activation(fs[:], rho[:], AF.Sin, scale=g)
                    fc = fp.tile([128, RPC], F16, name="fc")
                    nc.scalar.activation(fc[:], ar[:], AF.Sin, bias=halfpi[:],
                                         scale=-g)

                    start = pair == 0
                    stop = pair == npairs - 1
                    for ot in range(OT):
                        for nh in range(NH):
                            p = ps[ot * NH + nh]
                            nc.tensor.matmul(
                                p[:], wc[:, ot * 128:(ot + 1) * 128],
                                fc[:, nh * 512:(nh + 1) * 512],
                                start=start, stop=False)
                            nc.tensor.matmul(
                                p[:], ws[:, ot * 128:(ot + 1) * 128],
                                fs[:, nh * 512:(nh + 1) * 512],
                                start=False, stop=stop)
                    pair += 1

            for ot in range(OT):
                for nh in range(NH):
                    yt = yp.tile([128, 512], F32, name="yt")
                    nc.vector.tensor_scalar_add(yt[:], ps[ot * NH + nh][:],
                                                bt[ot][:])
                    nc.gpsimd.dma_start(
                        yT[ot * 128:(ot + 1) * 128, nh * 512:(nh + 1) * 512],
                        yt[:])

    _split_multiwaits(nc)
    return nc


def prep_inputs(x, fouriercoeffs, bias):
    ct = np.asarray(fouriercoeffs).transpose(0, 2, 3, 1)  # [2, I, G, O]
    W = ct.reshape(2, IB, 128, G, O).transpose(1, 3, 0, 2, 4)  # [IB,G,2,128,O]
    W = np.ascontiguousarray(W.reshape(KT, 128, O)).astype(np.float16)
    br = np.ascontiguousarray(np.asarray(bias, np.float32).reshape(OT, 128, 1))
    in_maps = []
    for c in range(NCORES):
        xTc = np.ascontiguousarray(np.asarray(x[c * RPC:(c + 1) * RPC]).T)
        in_maps.append({"xT": xTc, "w": W, "biasd": br})
    return in_maps


def kernel(x, fouriercoeffs, bias):
    nc = build()
    in_maps = prep_inputs(x, fouriercoeffs, bias)
    res = run_bass_kernel_spmd(nc, in_maps, core_ids=list(range(NCORES)))
    y = np.empty((N, O), np.float32)
    for c in range(NCORES):
        y[c * RPC:(c + 1) * RPC] = res.results[c]["yT"].T
    return y
